# revision 37
# baseline (speedup 1.0000x reference)
"""Trainium2 Bass kernel for a NonLocalBlock (B=2, C=256, H=W=64).

Math (reference):
    theta/phi/g = 1x1 conv of inp (C -> CH=128), L = CH*H = 8192
    attn = softmax(th @ ph^T) over kv, with th, ph (L, W=64)
    o = attn @ gg -> out = conv1x1(o) + b_out + inp

Sharding: 8 cores = 2 samples x 4 h-blocks (16 h rows each). Each core
computes the attention output rows q=(ch, h) for its h-block, which is
exactly the data the final conv needs for output columns (h, w) of its
block, so there is no cross-core reduction.

The per-core x is column-permuted so the core's own 16 h rows come
first: the kernel is then identical on every core (SPMD) — softmax over
kv is permutation-invariant as long as phT and V use the same h order,
and both are derived from the same permuted x.

Per-core layouts (kv order = (h', ch') so V tiles come straight from the
g conv output; q order = (h, ch) so attention output transposes directly
into (ch, (h, w)) for the out conv):
    Qt  (64=w,  2048=q)    phT (64=w, 8192=kv)    vaug (128=ch', 64*65)
    S^T tile t = phT[:, t*128:(t+1)*128].T @ Qt   -> psum (128=kv, q)
    E = exp(S^T)  (no max subtraction: logits are within +-80 in fp32)
    O'^T += E.T @ [V_t | 1]  -> psum (65, q): rows 0..63 = o, row 64 = denom

dtypes: Q/K/V fp16 (10-bit mantissa ~ fp32r accuracy, 1 cyc/row + fast
weight load), E bf16 (needs fp32 exponent range: E spans e^+-70), all
matmul accumulation fp32 in PSUM, logits fp32, out conv fp32r.
"""

import numpy as np

B, C, H, W = 2, 256, 64, 64
CH = C // 2          # 128
HS = H // 4          # 16 h rows per core
LQ = CH * HS         # 2048 q rows per core
NKV = 64             # kv tiles of 128 (kv = (h', ch'))
QP = 1024            # q per attention pass (PSUM budget)

# Schraudolph exp on the vector engine: bf16(e^x) ~= bitcast_bf16(
# int16(round(x*SA + SB))) -- one fused DVE tensor_scalar, which offloads
# part of the exp stream from the saturated scalar engine. SB is centred
# to split the piecewise-linear-mantissa error to +-3.3%.
SA = 184.6649652337873   # 2^7 * log2(e)
SB = 16250.4             # 127*2^7 - 5.6 (centred)


def _dve_tile(it):
    # 3-of-8 kv tiles take the approximate DVE exp; one op per tile keeps
    # the per-op overhead minimal while balancing the two engines' load
    return it % 8 in (1, 4, 6)

_cached = {}


def _build_nc():
    import concourse.mybir as mybir
    import concourse.tile as tile
    from concourse import bacc

    f32 = mybir.dt.float32
    f32r = mybir.dt.float32r
    f16 = mybir.dt.float16
    bf16 = mybir.dt.bfloat16
    i16 = mybir.dt.int16
    AF = mybir.ActivationFunctionType
    ALU = mybir.AluOpType

    nc = bacc.Bacc("TRN2", target_bir_lowering=False, debug=False, num_devices=8)

    x0 = nc.dram_tensor("x0", [128, 4096], f16, kind="ExternalInput")
    x1 = nc.dram_tensor("x1", [128, 4096], f16, kind="ExternalInput")
    xs0 = nc.dram_tensor("xs0", [128, 1024], f32, kind="ExternalInput")
    xs1 = nc.dram_tensor("xs1", [128, 1024], f32, kind="ExternalInput")
    wc = nc.dram_tensor("wc", [128, 2, 256], f16, kind="ExternalInput")
    wg = nc.dram_tensor("wg", [128, 2, 128], f16, kind="ExternalInput")
    wo = nc.dram_tensor("wo", [128, 2, 128], f16, kind="ExternalInput")
    bth = nc.dram_tensor("bth", [64, 128], f32, kind="ExternalInput")
    bph = nc.dram_tensor("bph", [64, 128], f32, kind="ExternalInput")
    bg = nc.dram_tensor("bg", [128, 1], f32, kind="ExternalInput")
    bo = nc.dram_tensor("bo", [128, 2], f32, kind="ExternalInput")
    ident = nc.dram_tensor("ident", [128, 128], f32, kind="ExternalInput")
    y = nc.dram_tensor("y", [2, 128, 1024], f32, kind="ExternalOutput")

    with tile.TileContext(nc) as tc:
        with (
            tc.tile_pool(name="const", bufs=1) as cp,
            tc.tile_pool(name="big", bufs=1) as bp,
            tc.tile_pool(name="work", bufs=3) as wkp,
            tc.tile_pool(name="psum", bufs=1, space="PSUM") as pp,
        ):
            x0t = bp.tile([128, 4096], f16, tag="x0")
            x1t = bp.tile([128, 4096], f16, tag="x1")
            xs0t = bp.tile([128, 1024], f32, tag="xs0")
            xs1t = bp.tile([128, 1024], f32, tag="xs1")
            # critical startup loads fan out over four independent DMA
            # queues in 512-col chunks: each queue moves ~116 GB/s, so the
            # first convs (cols 0:512) unblock ~1.1us after issue and the
            # first attention pass (cols 0:1024) ~2.2us after
            nc.sync.dma_start(out=x0t[:, 0:256], in_=x0[:, 0:256])
            nc.gpsimd.dma_start(out=x1t[:, 0:256], in_=x1[:, 0:256])
            nc.sync.dma_start(out=x0t[:, 256:512], in_=x0[:, 256:512])
            nc.gpsimd.dma_start(out=x1t[:, 256:512], in_=x1[:, 256:512])

            wct = cp.tile([128, 2, 256], f16, tag="wc")
            wgt = cp.tile([128, 2, 128], f16, tag="wg")
            wot = cp.tile([128, 2, 128], f16, tag="wo")
            btht = cp.tile([64, 128], f32, tag="bth")
            bpht = cp.tile([64, 128], f32, tag="bph")
            bgt = cp.tile([128, 1], f32, tag="bg")
            bot = cp.tile([128, 2], f32, tag="bo")
            idt = cp.tile([128, 128], f32, tag="ident")
            # only loads the prologue needs go on the queue up front; the
            # rest are emitted inside the main loop so the first h-pair's
            # DMA-semaphore wait clears as early as possible
            nc.scalar.dma_start(out=wct[:], in_=wc[:])
            nc.scalar.dma_start(out=btht[:], in_=bth[:])
            nc.scalar.dma_start(out=bpht[:], in_=bph[:])
            nc.scalar.dma_start(out=x0t[:, 512:1024], in_=x0[:, 512:1024])
            nc.gpsimd.dma_start(out=x1t[:, 512:1024], in_=x1[:, 512:1024])
            nc.gpsimd.dma_start(out=wgt[:], in_=wg[:])
            nc.sync.dma_start(out=bgt[:], in_=bg[:])
            for c0 in range(1024, 2048, 512):
                nc.sync.dma_start(out=x0t[:, c0:c0 + 512],
                                  in_=x0[:, c0:c0 + 512])
                nc.gpsimd.dma_start(out=x1t[:, c0:c0 + 512],
                                    in_=x1[:, c0:c0 + 512])

            def emit_late_dmas(step):
                if step == 0:
                    nc.sync.dma_start(out=x0t[:, 2048:4096],
                                      in_=x0[:, 2048:4096])
                    nc.sync.dma_start(out=x1t[:, 2048:4096],
                                      in_=x1[:, 2048:4096])
                elif step == 1:
                    nc.sync.dma_start(out=xs0t[:], in_=xs0[:])
                    nc.sync.dma_start(out=xs1t[:], in_=xs1[:])
                elif step == 2:
                    nc.sync.dma_start(out=wot[:], in_=wo[:])
                    nc.sync.dma_start(out=bot[:], in_=bo[:])
                    nc.sync.dma_start(out=idt[:], in_=ident[:])

            # preload the exp table set while DMAs/convs run, so the
            # first attention exp does not stall on ACT_TABLE_LOAD
            warm = wkp.tile([1, 1], f32, tag="warm")
            nc.scalar.activation(warm[:], btht[0:1, 0:1], AF.Exp)

            # [w; w] duplicated along partitions so the S matmul runs at
            # K=128: the HAM activity monitor never un-throttles the PE
            # clock (stays 1.2 GHz) for K=64 matmuls, measured directly.
            # The phi half of wc/bth is pre-scaled by 0.5 on the host so
            # the duplicated contraction sums to the original dot product.
            qt = bp.tile([128, LQ], f16, tag="qt")         # [Qt; Qt] (w2, q)
            pht = bp.tile([128, 8192], f16, tag="pht")     # [phT; phT]/2
            vaug = bp.tile([128, NKV * 65], bf16, tag="vaug")
            osb = bp.tile([128, 1024], f16, tag="osb")    # o (ch, (h, w))
            otsb = bp.tile([65, LQ], f32, tag="otsb")      # O'^T staged in SBUF
            ysb0 = bp.tile([128, 1024], f32, tag="ysb0")
            ysb1 = bp.tile([128, 1024], f32, tag="ysb1")
            xsb0 = bp.tile([128, 1024], f32, tag="xsb0")
            xsb1 = bp.tile([128, 1024], f32, tag="xsb1")
            vaug3 = vaug.rearrange("p (t j) -> p t j", j=65)
            nc.vector.memset(vaug3[:, :, 64:65], 1.0)

            # ---- producer emitters (interleaved into the attention loop so
            # the PE/DVE streams overlap attention instead of preceding it) --

            pht4 = pht.rearrange("p (g t c) -> p g t c", t=2, c=128)
            qt4 = qt.rearrange("p (g t c) -> p g t c", t=2, c=128)

            def emit_hp2(j):
                # Qt and phT columns for TWO h-pairs (hp = 2j, 2j+1) from x
                # via the [w_phi | w_theta] concatenated weights; batching two
                # h-pairs into one [128, 512] psum tile lets each evacuation
                # run as a single 3D-AP op over both (FD=256), halving the
                # DVE op count on the conv critical path
                hp0 = 2 * j
                n1 = 256 if hp0 < 8 else 128
                ps = pp.tile([128, 512], f32, tag="conv", bufs=2,
                             name=f"c{j}")
                for k in range(2):
                    hp = hp0 + k
                    nc.tensor.matmul(ps[:, k * 256:k * 256 + n1],
                                     lhsT=x0t[:, hp * 128:(hp + 1) * 128],
                                     rhs=wct[:, 0, 0:n1],
                                     start=True, stop=False)
                    nc.tensor.matmul(ps[:, k * 256:k * 256 + n1],
                                     lhsT=x1t[:, hp * 128:(hp + 1) * 128],
                                     rhs=wct[:, 1, 0:n1],
                                     start=False, stop=True)
                ps4 = ps.rearrange("p (k c) -> p k c", c=256)
                for hh in range(2):
                    # h = 2*hp + hh -> pht/qt col blocks at (4j+hh)*128,
                    # step 256 across the two hps of this batch
                    nc.vector.tensor_tensor(
                        out=pht4[0:64, 2 * j:2 * j + 2, hh, :],
                        in0=ps4[hh * 64:(hh + 1) * 64, :, 0:128],
                        in1=bpht[:].rearrange("p (o c) -> p o c", o=1)
                            .to_broadcast([64, 2, 128]),
                        op=ALU.add)
                    if hp0 < 8:
                        nc.vector.tensor_tensor(
                            out=qt4[0:64, 2 * j:2 * j + 2, hh, :],
                            in0=ps4[hh * 64:(hh + 1) * 64, :, 128:256],
                            in1=btht[:].rearrange("p (o c) -> p o c", o=1)
                                .to_broadcast([64, 2, 128]),
                            op=ALU.add)
                # duplicated w rows (partitions 64..127) via DMA -- no
                # compute-engine time; gpsimd's queue drains fast once the
                # x chunk stream finishes. Tiles 4j..4j+3 of pht.
                if j == 1:
                    nc.gpsimd.dma_start(out=qt[64:128, 0:1024],
                                        in_=qt[0:64, 0:1024])
                if j == 3:
                    nc.gpsimd.dma_start(out=qt[64:128, 1024:2048],
                                        in_=qt[0:64, 1024:2048])
                if j >= 1:
                    nc.gpsimd.dma_start(out=pht[64:128, j * 512:(j + 1) * 512],
                                        in_=pht[0:64, j * 512:(j + 1) * 512])

            def emit_g(n):
                # g conv chunk -> vaug (values only; ones column pre-memset)
                ps = pp.tile([128, 512], f32, tag="conv", bufs=2,
                             name=f"g{n}")
                nc.tensor.matmul(ps[:], lhsT=wgt[:, 0, :],
                                 rhs=x0t[:, n * 512:(n + 1) * 512],
                                 start=True, stop=False)
                nc.tensor.matmul(ps[:], lhsT=wgt[:, 1, :],
                                 rhs=x1t[:, n * 512:(n + 1) * 512],
                                 start=False, stop=True)
                nc.vector.tensor_scalar(
                    out=vaug3[:, n * 8:(n + 1) * 8, 0:64],
                    in0=ps.rearrange("p (t j) -> p t j", j=64)[:],
                    scalar1=bgt[:, 0:1], scalar2=None, op0=ALU.add)

            def emit_lh(lh, on_act):
                # transpose + normalize one 128-q block of O'^T
                trp = pp.tile([128, 65], f32, tag="conv", bufs=2,
                              name=f"tr{lh}")
                nc.tensor.transpose(trp[:],
                                    otsb[:, lh * 128:(lh + 1) * 128],
                                    idt[0:65, 0:65])
                rden = wkp.tile([128, 1], f32, tag="rden", name=f"rd{lh}")
                nc.vector.reciprocal(rden[:], trp[:, 64:65])
                if on_act:
                    nc.scalar.activation(osb[:, lh * 64:(lh + 1) * 64],
                                         trp[:, 0:64], AF.Copy,
                                         scale=rden[:])
                else:
                    nc.vector.tensor_scalar(
                        out=osb[:, lh * 64:(lh + 1) * 64],
                        in0=trp[:, 0:64],
                        scalar1=rden[:], scalar2=None, op0=ALU.mult)

            def emit_xsb(m):
                # precombine residual + out-conv bias while attention runs
                xsb = xsb0 if m == 0 else xsb1
                xres = xs0t if m == 0 else xs1t
                nc.vector.tensor_scalar(out=xsb[:], in0=xres[:],
                                        scalar1=bot[:, m:m + 1], scalar2=None,
                                        op0=ALU.add)

            def emit_y(m, c0, c1):
                # out conv for columns [c0:c1] + (bias+residual) + store
                ysb = ysb0 if m == 0 else ysb1
                xsb = xsb0 if m == 0 else xsb1
                yp = pp.tile([128, 512], f32, tag="s", bufs=2,
                             name=f"yp{m}{c0}")
                nc.tensor.matmul(yp[:, 0:c1 - c0], lhsT=wot[:, m, :],
                                 rhs=osb[:, c0:c1], start=True, stop=True)
                nc.vector.tensor_tensor(
                    out=ysb[:, c0:c1], in0=yp[:, 0:c1 - c0],
                    in1=xsb[:, c0:c1], op=ALU.add)
                nc.sync.dma_start(out=y[m, :, c0:c1], in_=ysb[:, c0:c1])

            # ---- attention: software-pipelined over 2 q passes of 1024 ----
            NIT = 2 * NKV
            ets = {}
            otps = {}

            def emit_s(it):
                p, t = it // NKV, it % NKV
                # kv tiles 0-3 contract K=64 (no duplicated w rows -- those
                # rows aren't ready during startup); since phi is pre-halved
                # on the host for the K=128 duplicated contraction, the K=64
                # logits come out at 0.5x and the exp scale compensates.
                kk = 64 if t < 4 else 128
                sp = pp.tile([128, QP], f32, tag="s", bufs=2, name=f"sp{it}")
                for c in range(2):
                    nc.tensor.matmul(
                        sp[:, c * 512:(c + 1) * 512],
                        lhsT=pht[0:kk, t * 128:(t + 1) * 128],
                        rhs=qt[0:kk, p * QP + c * 512: p * QP + (c + 1) * 512],
                        start=True, stop=True)
                et = wkp.tile([128, QP], bf16, tag="e", bufs=4,
                              name=f"et{it}")
                esc = 2.0 if kk == 64 else 1.0
                if _dve_tile(it):
                    nc.vector.tensor_scalar(
                        out=et[:].bitcast(i16), in0=sp[:],
                        scalar1=SA * esc, scalar2=SB,
                        op0=ALU.mult, op1=ALU.add)
                else:
                    nc.scalar.activation(et[:], sp[:], AF.Exp, scale=esc)
                ets[it] = et

            def emit_pv(it):
                p, t = it // NKV, it % NKV
                if t == 0:
                    otps[p] = pp.tile([65, QP], f32, tag="ot", bufs=1,
                                      name=f"otp{p}")
                otp = otps[p]
                et = ets.pop(it)
                for c in range(2):
                    nc.tensor.matmul(
                        otp[:, c * 512:(c + 1) * 512],
                        lhsT=vaug3[:, t, :],
                        rhs=et[:, c * 512:(c + 1) * 512],
                        start=(t == 0), stop=(t == NKV - 1),
                        skip_group_check=True)
                if t == NKV - 1:
                    # evacuate O'^T in 256-col chunks on both free-ish
                    # engines so the downstream transpose/normalize chain
                    # starts as soon as its slice lands
                    for dc in range(4):
                        dst = otsb[:, p * QP + dc * 256:p * QP + (dc + 1) * 256]
                        src = otp[:, dc * 256:(dc + 1) * 256]
                        if dc % 2 == 0:
                            nc.vector.tensor_copy(dst, src)
                        else:
                            nc.scalar.activation(dst, src, AF.Copy)
                    if p == 0:
                        for lh in range(8):
                            todo.append(lambda lh=lh: emit_lh(lh, False))
                        todo.append(lambda: emit_xsb(0))
                        todo.append(lambda: emit_xsb(1))
                        todo.append(lambda: emit_y(0, 0, 512))
                        todo.append(lambda: emit_y(1, 0, 512))

            from collections import deque
            todo = deque()
            n_j = 2
            n_g = 1
            for j in range(n_j):
                emit_hp2(j)
            emit_s(0)
            emit_s(1)
            emit_g(0)
            for it in range(2, NIT):
                emit_s(it)
                emit_pv(it - 2)
                if it in (2, 4, 6):
                    emit_late_dmas(it // 2 - 1)
                if it % 3 == 0 and n_j < 16:
                    emit_hp2(n_j)
                    n_j += 1
                if it % 6 == 5 and n_g < 8:
                    emit_g(n_g)
                    n_g += 1
                if todo and it % 2 == 1:
                    todo.popleft()()
            emit_pv(NIT - 2)
            emit_pv(NIT - 1)
            while todo:
                todo.popleft()()
            for lh in range(8, 16):
                emit_lh(lh, True)
                if lh == 11:
                    emit_y(0, 512, 768)
                    emit_y(1, 512, 768)
            emit_y(0, 768, 896)
            emit_y(1, 768, 896)
            emit_y(0, 896, 1024)
            emit_y(1, 896, 1024)

    nc.compile()
    return nc


def _get_nc():
    if "nc" not in _cached:
        _cached["nc"] = _build_nc()
    return _cached["nc"]


LAST_EXEC_NS = None
LAST_TRACE_DIR = None


def kernel(inp, w_theta, b_theta, w_phi, b_phi, w_g, b_g, w_out, b_out):
    import os
    from concourse.bass_utils import run_bass_kernel_spmd

    nc = _get_nc()

    f = np.float32
    c = np.ascontiguousarray

    # [w_phi | w_theta] concatenated, as (c_lo, half, 256) fp16
    wcat = np.concatenate([w_phi.T * 0.5, w_theta.T], axis=1).astype(f)
    wc3 = c(wcat.reshape(2, 128, 256).transpose(1, 0, 2).astype(np.float16))
    wg3 = c(w_g.T.reshape(2, 128, CH).transpose(1, 0, 2).astype(np.float16))
    wo3 = c(w_out.reshape(2, 128, CH).transpose(2, 0, 1).astype(np.float16))  # [ch, m, co]
    bth1 = c(np.tile(b_theta.astype(f), (64, 1)))
    bph1 = c(np.tile(b_phi.astype(f) * 0.5, (64, 1)))
    bg1 = c(b_g.astype(f)[:, None])
    bo2 = c(b_out.reshape(2, 128).T.astype(f))
    ident = np.eye(128, dtype=f)

    in_maps = []
    for core in range(8):
        b, k = core // 4, core % 4
        x = inp[b].reshape(C, H, W).astype(f)
        # own h-block first, then the rest: kernel is h-order agnostic
        perm = list(range(HS * k, HS * (k + 1))) + \
            [h for h in range(H) if not (HS * k <= h < HS * (k + 1))]
        xp = x[:, perm, :].reshape(C, H * W)
        xp16 = xp.astype(np.float16)
        in_maps.append({
            "x0": c(xp16[:128]), "x1": c(xp16[128:]),
            "xs0": c(xp[:128, :1024]), "xs1": c(xp[128:, :1024]),
            "wc": wc3, "wg": wg3, "wo": wo3,
            "bth": bth1, "bph": bph1, "bg": bg1, "bo": bo2, "ident": ident,
        })

    trace = bool(os.environ.get("NLB_TRACE"))
    tmpdir = os.environ.get("NLB_TRACE_DIR") or None
    res = run_bass_kernel_spmd(nc, in_maps, list(range(8)), trace=trace,
                               tmpdir=tmpdir)
    global LAST_EXEC_NS, LAST_TRACE_DIR
    LAST_EXEC_NS = res.exec_time_ns
    LAST_TRACE_DIR = tmpdir

    out = np.empty((B, C, H, W), dtype=f)
    for core in range(8):
        b, k = core // 4, core % 4
        yc = res.results[core]["y"].reshape(C, HS, W)
        out[b, :, HS * k:HS * (k + 1), :] = yc
    return out



# revision 38
# speedup vs baseline: 1.0121x; 1.0121x over previous
"""Trainium2 Bass kernel for a NonLocalBlock (B=2, C=256, H=W=64).

Math (reference):
    theta/phi/g = 1x1 conv of inp (C -> CH=128), L = CH*H = 8192
    attn = softmax(th @ ph^T) over kv, with th, ph (L, W=64)
    o = attn @ gg -> out = conv1x1(o) + b_out + inp

Sharding: 8 cores = 2 samples x 4 h-blocks (16 h rows each). Each core
computes the attention output rows q=(ch, h) for its h-block, which is
exactly the data the final conv needs for output columns (h, w) of its
block, so there is no cross-core reduction.

The per-core x is column-permuted so the core's own 16 h rows come
first: the kernel is then identical on every core (SPMD) — softmax over
kv is permutation-invariant as long as phT and V use the same h order,
and both are derived from the same permuted x.

Per-core layouts (kv order = (h', ch') so V tiles come straight from the
g conv output; q order = (h, ch) so attention output transposes directly
into (ch, (h, w)) for the out conv):
    Qt  (64=w,  2048=q)    phT (64=w, 8192=kv)    vaug (128=ch', 64*65)
    S^T tile t = phT[:, t*128:(t+1)*128].T @ Qt   -> psum (128=kv, q)
    E = exp(S^T)  (no max subtraction: logits are within +-80 in fp32)
    O'^T += E.T @ [V_t | 1]  -> psum (65, q): rows 0..63 = o, row 64 = denom

dtypes: Q/K/V fp16 (10-bit mantissa ~ fp32r accuracy, 1 cyc/row + fast
weight load), E bf16 (needs fp32 exponent range: E spans e^+-70), all
matmul accumulation fp32 in PSUM, logits fp32, out conv fp32r.
"""

import numpy as np

B, C, H, W = 2, 256, 64, 64
CH = C // 2          # 128
HS = H // 4          # 16 h rows per core
LQ = CH * HS         # 2048 q rows per core
NKV = 64             # kv tiles of 128 (kv = (h', ch'))
QP = 1024            # q per attention pass (PSUM budget)

# Schraudolph exp on the vector engine: bf16(e^x) ~= bitcast_bf16(
# int16(round(x*SA + SB))) -- one fused DVE tensor_scalar, which offloads
# part of the exp stream from the saturated scalar engine. SB is centred
# to split the piecewise-linear-mantissa error to +-3.3%.
SA = 184.6649652337873   # 2^7 * log2(e)
SB = 16250.4             # 127*2^7 - 5.6 (centred)


def _dve_tile(it):
    # 3-of-8 kv tiles take the approximate DVE exp; one op per tile keeps
    # the per-op overhead minimal while balancing the two engines' load
    return it % 8 in (1, 4, 6)

_cached = {}


def _build_nc():
    import concourse.mybir as mybir
    import concourse.tile as tile
    from concourse import bacc

    f32 = mybir.dt.float32
    f32r = mybir.dt.float32r
    f16 = mybir.dt.float16
    bf16 = mybir.dt.bfloat16
    i16 = mybir.dt.int16
    AF = mybir.ActivationFunctionType
    ALU = mybir.AluOpType

    nc = bacc.Bacc("TRN2", target_bir_lowering=False, debug=False, num_devices=8)

    x0 = nc.dram_tensor("x0", [128, 4096], f16, kind="ExternalInput")
    x1 = nc.dram_tensor("x1", [128, 4096], f16, kind="ExternalInput")
    xs0 = nc.dram_tensor("xs0", [128, 1024], f32, kind="ExternalInput")
    xs1 = nc.dram_tensor("xs1", [128, 1024], f32, kind="ExternalInput")
    wc = nc.dram_tensor("wc", [128, 2, 256], f16, kind="ExternalInput")
    wg = nc.dram_tensor("wg", [128, 2, 128], f16, kind="ExternalInput")
    wo = nc.dram_tensor("wo", [128, 2, 128], f16, kind="ExternalInput")
    bth = nc.dram_tensor("bth", [64, 128], f32, kind="ExternalInput")
    bph = nc.dram_tensor("bph", [64, 128], f32, kind="ExternalInput")
    bg = nc.dram_tensor("bg", [128, 1], f32, kind="ExternalInput")
    bo = nc.dram_tensor("bo", [128, 2], f32, kind="ExternalInput")
    ident = nc.dram_tensor("ident", [128, 128], f32, kind="ExternalInput")
    y = nc.dram_tensor("y", [2, 128, 1024], f32, kind="ExternalOutput")

    with tile.TileContext(nc) as tc:
        with (
            tc.tile_pool(name="const", bufs=1) as cp,
            tc.tile_pool(name="big", bufs=1) as bp,
            tc.tile_pool(name="work", bufs=3) as wkp,
            tc.tile_pool(name="psum", bufs=1, space="PSUM") as pp,
        ):
            x0t = bp.tile([128, 4096], f16, tag="x0")
            x1t = bp.tile([128, 4096], f16, tag="x1")
            xs0t = bp.tile([128, 1024], f32, tag="xs0")
            xs1t = bp.tile([128, 1024], f32, tag="xs1")
            # critical startup loads fan out over four independent DMA
            # queues in 512-col chunks: each queue moves ~116 GB/s, so the
            # first convs (cols 0:512) unblock ~1.1us after issue and the
            # first attention pass (cols 0:1024) ~2.2us after
            nc.sync.dma_start(out=x0t[:, 0:512], in_=x0[:, 0:512])
            nc.gpsimd.dma_start(out=x1t[:, 0:512], in_=x1[:, 0:512])

            wct = cp.tile([128, 2, 256], f16, tag="wc")
            wgt = cp.tile([128, 2, 128], f16, tag="wg")
            wot = cp.tile([128, 2, 128], f16, tag="wo")
            btht = cp.tile([64, 128], f32, tag="bth")
            bpht = cp.tile([64, 128], f32, tag="bph")
            bgt = cp.tile([128, 1], f32, tag="bg")
            bot = cp.tile([128, 2], f32, tag="bo")
            idt = cp.tile([128, 128], f32, tag="ident")
            # only loads the prologue needs go on the queue up front; the
            # rest are emitted inside the main loop so the first h-pair's
            # DMA-semaphore wait clears as early as possible
            nc.scalar.dma_start(out=wct[:], in_=wc[:])
            nc.scalar.dma_start(out=btht[:], in_=bth[:])
            nc.scalar.dma_start(out=bpht[:], in_=bph[:])
            nc.scalar.dma_start(out=x0t[:, 512:1024], in_=x0[:, 512:1024])
            nc.gpsimd.dma_start(out=x1t[:, 512:1024], in_=x1[:, 512:1024])
            nc.gpsimd.dma_start(out=wgt[:], in_=wg[:])
            nc.sync.dma_start(out=bgt[:], in_=bg[:])
            for c0 in range(1024, 2048, 512):
                nc.sync.dma_start(out=x0t[:, c0:c0 + 512],
                                  in_=x0[:, c0:c0 + 512])
                nc.gpsimd.dma_start(out=x1t[:, c0:c0 + 512],
                                    in_=x1[:, c0:c0 + 512])

            def emit_late_dmas(step):
                if step == 0:
                    nc.sync.dma_start(out=x0t[:, 2048:4096],
                                      in_=x0[:, 2048:4096])
                    nc.sync.dma_start(out=x1t[:, 2048:4096],
                                      in_=x1[:, 2048:4096])
                elif step == 1:
                    nc.sync.dma_start(out=xs0t[:], in_=xs0[:])
                    nc.sync.dma_start(out=xs1t[:], in_=xs1[:])
                elif step == 2:
                    nc.sync.dma_start(out=wot[:], in_=wo[:])
                    nc.sync.dma_start(out=bot[:], in_=bo[:])
                    nc.sync.dma_start(out=idt[:], in_=ident[:])

            # preload the exp table set while DMAs/convs run, so the
            # first attention exp does not stall on ACT_TABLE_LOAD
            warm = wkp.tile([1, 1], f32, tag="warm")
            nc.scalar.activation(warm[:], btht[0:1, 0:1], AF.Exp)

            # [w; w] duplicated along partitions so the S matmul runs at
            # K=128: the HAM activity monitor never un-throttles the PE
            # clock (stays 1.2 GHz) for K=64 matmuls, measured directly.
            # The phi half of wc/bth is pre-scaled by 0.5 on the host so
            # the duplicated contraction sums to the original dot product.
            qt = bp.tile([128, LQ], f16, tag="qt")         # [Qt; Qt] (w2, q)
            pht = bp.tile([128, 8192], f16, tag="pht")     # [phT; phT]/2
            vaug = bp.tile([128, NKV * 65], bf16, tag="vaug")
            osb = bp.tile([128, 1024], f16, tag="osb")    # o (ch, (h, w))
            otsb = bp.tile([65, LQ], f32, tag="otsb")      # O'^T staged in SBUF
            ysb0 = bp.tile([128, 1024], f32, tag="ysb0")
            ysb1 = bp.tile([128, 1024], f32, tag="ysb1")
            xsb0 = bp.tile([128, 1024], f32, tag="xsb0")
            xsb1 = bp.tile([128, 1024], f32, tag="xsb1")
            vaug3 = vaug.rearrange("p (t j) -> p t j", j=65)
            nc.vector.memset(vaug3[:, :, 64:65], 1.0)

            # ---- producer emitters (interleaved into the attention loop so
            # the PE/DVE streams overlap attention instead of preceding it) --

            pht4 = pht.rearrange("p (g t c) -> p g t c", t=2, c=128)
            qt4 = qt.rearrange("p (g t c) -> p g t c", t=2, c=128)

            def emit_hp2(j):
                # Qt and phT columns for TWO h-pairs (hp = 2j, 2j+1) from x
                # via the [w_phi | w_theta] concatenated weights; batching two
                # h-pairs into one [128, 512] psum tile lets each evacuation
                # run as a single 3D-AP op over both (FD=256), halving the
                # DVE op count on the conv critical path
                hp0 = 2 * j
                n1 = 256 if hp0 < 8 else 128
                ps = pp.tile([128, 512], f32, tag="conv", bufs=2,
                             name=f"c{j}")
                for k in range(2):
                    hp = hp0 + k
                    nc.tensor.matmul(ps[:, k * 256:k * 256 + n1],
                                     lhsT=x0t[:, hp * 128:(hp + 1) * 128],
                                     rhs=wct[:, 0, 0:n1],
                                     start=True, stop=False)
                    nc.tensor.matmul(ps[:, k * 256:k * 256 + n1],
                                     lhsT=x1t[:, hp * 128:(hp + 1) * 128],
                                     rhs=wct[:, 1, 0:n1],
                                     start=False, stop=True)
                ps4 = ps.rearrange("p (k c) -> p k c", c=256)
                for hh in range(2):
                    # h = 2*hp + hh -> pht/qt col blocks at (4j+hh)*128,
                    # step 256 across the two hps of this batch
                    nc.vector.tensor_tensor(
                        out=pht4[0:64, 2 * j:2 * j + 2, hh, :],
                        in0=ps4[hh * 64:(hh + 1) * 64, :, 0:128],
                        in1=bpht[:].rearrange("p (o c) -> p o c", o=1)
                            .to_broadcast([64, 2, 128]),
                        op=ALU.add)
                    if hp0 < 8:
                        nc.vector.tensor_tensor(
                            out=qt4[0:64, 2 * j:2 * j + 2, hh, :],
                            in0=ps4[hh * 64:(hh + 1) * 64, :, 128:256],
                            in1=btht[:].rearrange("p (o c) -> p o c", o=1)
                                .to_broadcast([64, 2, 128]),
                            op=ALU.add)
                # duplicated w rows (partitions 64..127) via DMA -- no
                # compute-engine time; gpsimd's queue drains fast once the
                # x chunk stream finishes. Tiles 4j..4j+3 of pht.
                if j == 1:
                    nc.gpsimd.dma_start(out=qt[64:128, 0:1024],
                                        in_=qt[0:64, 0:1024])
                if j == 3:
                    nc.gpsimd.dma_start(out=qt[64:128, 1024:2048],
                                        in_=qt[0:64, 1024:2048])
                if j >= 1:
                    nc.gpsimd.dma_start(out=pht[64:128, j * 512:(j + 1) * 512],
                                        in_=pht[0:64, j * 512:(j + 1) * 512])

            def emit_g(n):
                # g conv chunk -> vaug (values only; ones column pre-memset)
                ps = pp.tile([128, 512], f32, tag="conv", bufs=2,
                             name=f"g{n}")
                nc.tensor.matmul(ps[:], lhsT=wgt[:, 0, :],
                                 rhs=x0t[:, n * 512:(n + 1) * 512],
                                 start=True, stop=False)
                nc.tensor.matmul(ps[:], lhsT=wgt[:, 1, :],
                                 rhs=x1t[:, n * 512:(n + 1) * 512],
                                 start=False, stop=True)
                nc.vector.tensor_scalar(
                    out=vaug3[:, n * 8:(n + 1) * 8, 0:64],
                    in0=ps.rearrange("p (t j) -> p t j", j=64)[:],
                    scalar1=bgt[:, 0:1], scalar2=None, op0=ALU.add)

            def emit_lh(lh, on_act):
                # transpose + normalize one 128-q block of O'^T
                trp = pp.tile([128, 65], f32, tag="conv", bufs=2,
                              name=f"tr{lh}")
                nc.tensor.transpose(trp[:],
                                    otsb[:, lh * 128:(lh + 1) * 128],
                                    idt[0:65, 0:65])
                rden = wkp.tile([128, 1], f32, tag="rden", name=f"rd{lh}")
                nc.vector.reciprocal(rden[:], trp[:, 64:65])
                if on_act:
                    nc.scalar.activation(osb[:, lh * 64:(lh + 1) * 64],
                                         trp[:, 0:64], AF.Copy,
                                         scale=rden[:])
                else:
                    nc.vector.tensor_scalar(
                        out=osb[:, lh * 64:(lh + 1) * 64],
                        in0=trp[:, 0:64],
                        scalar1=rden[:], scalar2=None, op0=ALU.mult)

            def emit_xsb(m):
                # precombine residual + out-conv bias while attention runs
                xsb = xsb0 if m == 0 else xsb1
                xres = xs0t if m == 0 else xs1t
                nc.vector.tensor_scalar(out=xsb[:], in0=xres[:],
                                        scalar1=bot[:, m:m + 1], scalar2=None,
                                        op0=ALU.add)

            def emit_y(m, c0, c1):
                # out conv for columns [c0:c1] + (bias+residual) + store
                ysb = ysb0 if m == 0 else ysb1
                xsb = xsb0 if m == 0 else xsb1
                yp = pp.tile([128, 512], f32, tag="s", bufs=2,
                             name=f"yp{m}{c0}")
                nc.tensor.matmul(yp[:, 0:c1 - c0], lhsT=wot[:, m, :],
                                 rhs=osb[:, c0:c1], start=True, stop=True)
                nc.vector.tensor_tensor(
                    out=ysb[:, c0:c1], in0=yp[:, 0:c1 - c0],
                    in1=xsb[:, c0:c1], op=ALU.add)
                nc.sync.dma_start(out=y[m, :, c0:c1], in_=ysb[:, c0:c1])

            # ---- attention: software-pipelined over 2 q passes of 1024 ----
            NIT = 2 * NKV
            ets = {}
            otps = {}

            def emit_s(it):
                p, t = it // NKV, it % NKV
                # kv tiles 0-3 contract K=64 (no duplicated w rows -- those
                # rows aren't ready during startup); since phi is pre-halved
                # on the host for the K=128 duplicated contraction, the K=64
                # logits come out at 0.5x and the exp scale compensates.
                kk = 64 if t < 4 else 128
                sp = pp.tile([128, QP], f32, tag="s", bufs=2, name=f"sp{it}")
                for c in range(2):
                    nc.tensor.matmul(
                        sp[:, c * 512:(c + 1) * 512],
                        lhsT=pht[0:kk, t * 128:(t + 1) * 128],
                        rhs=qt[0:kk, p * QP + c * 512: p * QP + (c + 1) * 512],
                        start=True, stop=True)
                et = wkp.tile([128, QP], bf16, tag="e", bufs=4,
                              name=f"et{it}")
                esc = 2.0 if kk == 64 else 1.0
                if _dve_tile(it):
                    nc.vector.tensor_scalar(
                        out=et[:].bitcast(i16), in0=sp[:],
                        scalar1=SA * esc, scalar2=SB,
                        op0=ALU.mult, op1=ALU.add)
                else:
                    nc.scalar.activation(et[:], sp[:], AF.Exp, scale=esc)
                ets[it] = et

            def emit_pv(it):
                p, t = it // NKV, it % NKV
                if t == 0:
                    otps[p] = pp.tile([65, QP], f32, tag="ot", bufs=1,
                                      name=f"otp{p}")
                otp = otps[p]
                et = ets.pop(it)
                for c in range(2):
                    nc.tensor.matmul(
                        otp[:, c * 512:(c + 1) * 512],
                        lhsT=vaug3[:, t, :],
                        rhs=et[:, c * 512:(c + 1) * 512],
                        start=(t == 0), stop=(t == NKV - 1),
                        skip_group_check=True)
                if t == NKV - 1:
                    # evacuate O'^T in 256-col chunks on both free-ish
                    # engines so the downstream transpose/normalize chain
                    # starts as soon as its slice lands
                    for dc in range(4):
                        dst = otsb[:, p * QP + dc * 256:p * QP + (dc + 1) * 256]
                        src = otp[:, dc * 256:(dc + 1) * 256]
                        if dc % 2 == 0:
                            nc.vector.tensor_copy(dst, src)
                        else:
                            nc.scalar.activation(dst, src, AF.Copy)
                    if p == 0:
                        for lh in range(8):
                            todo.append(lambda lh=lh: emit_lh(lh, False))
                        todo.append(lambda: emit_xsb(0))
                        todo.append(lambda: emit_xsb(1))
                        todo.append(lambda: emit_y(0, 0, 512))
                        todo.append(lambda: emit_y(1, 0, 512))

            from collections import deque
            todo = deque()
            n_j = 2
            n_g = 1
            for j in range(n_j):
                emit_hp2(j)
            emit_s(0)
            emit_s(1)
            emit_g(0)
            for it in range(2, NIT):
                emit_s(it)
                emit_pv(it - 2)
                if it in (2, 4, 6):
                    emit_late_dmas(it // 2 - 1)
                if it % 3 == 0 and n_j < 16:
                    emit_hp2(n_j)
                    n_j += 1
                if it % 6 == 5 and n_g < 8:
                    emit_g(n_g)
                    n_g += 1
                if todo and it % 2 == 1:
                    todo.popleft()()
            emit_pv(NIT - 2)
            emit_pv(NIT - 1)
            while todo:
                todo.popleft()()
            for lh in range(8, 16):
                emit_lh(lh, True)
                if lh == 11:
                    emit_y(0, 512, 768)
                    emit_y(1, 512, 768)
            emit_y(0, 768, 1024)
            emit_y(1, 768, 1024)

    nc.compile()
    return nc


def _get_nc():
    if "nc" not in _cached:
        _cached["nc"] = _build_nc()
    return _cached["nc"]


LAST_EXEC_NS = None
LAST_TRACE_DIR = None


def kernel(inp, w_theta, b_theta, w_phi, b_phi, w_g, b_g, w_out, b_out):
    import os
    from concourse.bass_utils import run_bass_kernel_spmd

    nc = _get_nc()

    f = np.float32
    c = np.ascontiguousarray

    # [w_phi | w_theta] concatenated, as (c_lo, half, 256) fp16
    wcat = np.concatenate([w_phi.T * 0.5, w_theta.T], axis=1).astype(f)
    wc3 = c(wcat.reshape(2, 128, 256).transpose(1, 0, 2).astype(np.float16))
    wg3 = c(w_g.T.reshape(2, 128, CH).transpose(1, 0, 2).astype(np.float16))
    wo3 = c(w_out.reshape(2, 128, CH).transpose(2, 0, 1).astype(np.float16))  # [ch, m, co]
    bth1 = c(np.tile(b_theta.astype(f), (64, 1)))
    bph1 = c(np.tile(b_phi.astype(f) * 0.5, (64, 1)))
    bg1 = c(b_g.astype(f)[:, None])
    bo2 = c(b_out.reshape(2, 128).T.astype(f))
    ident = np.eye(128, dtype=f)

    in_maps = []
    for core in range(8):
        b, k = core // 4, core % 4
        x = inp[b].reshape(C, H, W).astype(f)
        # own h-block first, then the rest: kernel is h-order agnostic
        perm = list(range(HS * k, HS * (k + 1))) + \
            [h for h in range(H) if not (HS * k <= h < HS * (k + 1))]
        xp = x[:, perm, :].reshape(C, H * W)
        xp16 = xp.astype(np.float16)
        in_maps.append({
            "x0": c(xp16[:128]), "x1": c(xp16[128:]),
            "xs0": c(xp[:128, :1024]), "xs1": c(xp[128:, :1024]),
            "wc": wc3, "wg": wg3, "wo": wo3,
            "bth": bth1, "bph": bph1, "bg": bg1, "bo": bo2, "ident": ident,
        })

    trace = bool(os.environ.get("NLB_TRACE"))
    tmpdir = os.environ.get("NLB_TRACE_DIR") or None
    res = run_bass_kernel_spmd(nc, in_maps, list(range(8)), trace=trace,
                               tmpdir=tmpdir)
    global LAST_EXEC_NS, LAST_TRACE_DIR
    LAST_EXEC_NS = res.exec_time_ns
    LAST_TRACE_DIR = tmpdir

    out = np.empty((B, C, H, W), dtype=f)
    for core in range(8):
        b, k = core // 4, core % 4
        yc = res.results[core]["y"].reshape(C, HS, W)
        out[b, :, HS * k:HS * (k + 1), :] = yc
    return out



# revision 39
# speedup vs baseline: 1.0158x; 1.0037x over previous
"""Trainium2 Bass kernel for a NonLocalBlock (B=2, C=256, H=W=64).

Math (reference):
    theta/phi/g = 1x1 conv of inp (C -> CH=128), L = CH*H = 8192
    attn = softmax(th @ ph^T) over kv, with th, ph (L, W=64)
    o = attn @ gg -> out = conv1x1(o) + b_out + inp

Sharding: 8 cores = 2 samples x 4 h-blocks (16 h rows each). Each core
computes the attention output rows q=(ch, h) for its h-block, which is
exactly the data the final conv needs for output columns (h, w) of its
block, so there is no cross-core reduction.

The per-core x is column-permuted so the core's own 16 h rows come
first: the kernel is then identical on every core (SPMD) — softmax over
kv is permutation-invariant as long as phT and V use the same h order,
and both are derived from the same permuted x.

Per-core layouts (kv order = (h', ch') so V tiles come straight from the
g conv output; q order = (h, ch) so attention output transposes directly
into (ch, (h, w)) for the out conv):
    Qt  (64=w,  2048=q)    phT (64=w, 8192=kv)    vaug (128=ch', 64*65)
    S^T tile t = phT[:, t*128:(t+1)*128].T @ Qt   -> psum (128=kv, q)
    E = exp(S^T)  (no max subtraction: logits are within +-80 in fp32)
    O'^T += E.T @ [V_t | 1]  -> psum (65, q): rows 0..63 = o, row 64 = denom

dtypes: Q/K/V fp16 (10-bit mantissa ~ fp32r accuracy, 1 cyc/row + fast
weight load), E bf16 (needs fp32 exponent range: E spans e^+-70), all
matmul accumulation fp32 in PSUM, logits fp32, out conv fp32r.
"""

import numpy as np

B, C, H, W = 2, 256, 64, 64
CH = C // 2          # 128
HS = H // 4          # 16 h rows per core
LQ = CH * HS         # 2048 q rows per core
NKV = 64             # kv tiles of 128 (kv = (h', ch'))
QP = 1024            # q per attention pass (PSUM budget)

# Schraudolph exp on the vector engine: bf16(e^x) ~= bitcast_bf16(
# int16(round(x*SA + SB))) -- one fused DVE tensor_scalar, which offloads
# part of the exp stream from the saturated scalar engine. SB is centred
# to split the piecewise-linear-mantissa error to +-3.3%.
SA = 184.6649652337873   # 2^7 * log2(e)
SB = 16250.4             # 127*2^7 - 5.6 (centred)


def _dve_tile(it):
    # 3-of-8 kv tiles take the approximate DVE exp; one op per tile keeps
    # the per-op overhead minimal while balancing the two engines' load
    return it % 8 in (1, 4, 6)

_cached = {}


def _build_nc():
    import concourse.mybir as mybir
    import concourse.tile as tile
    from concourse import bacc

    f32 = mybir.dt.float32
    f32r = mybir.dt.float32r
    f16 = mybir.dt.float16
    bf16 = mybir.dt.bfloat16
    i16 = mybir.dt.int16
    AF = mybir.ActivationFunctionType
    ALU = mybir.AluOpType

    nc = bacc.Bacc("TRN2", target_bir_lowering=False, debug=False, num_devices=8)

    x0 = nc.dram_tensor("x0", [128, 4096], f16, kind="ExternalInput")
    x1 = nc.dram_tensor("x1", [128, 4096], f16, kind="ExternalInput")
    xs0 = nc.dram_tensor("xs0", [128, 1024], f32, kind="ExternalInput")
    xs1 = nc.dram_tensor("xs1", [128, 1024], f32, kind="ExternalInput")
    wc = nc.dram_tensor("wc", [128, 2, 256], f16, kind="ExternalInput")
    wg = nc.dram_tensor("wg", [128, 2, 128], f16, kind="ExternalInput")
    wo = nc.dram_tensor("wo", [128, 2, 128], f16, kind="ExternalInput")
    bth = nc.dram_tensor("bth", [64, 128], f32, kind="ExternalInput")
    bph = nc.dram_tensor("bph", [64, 128], f32, kind="ExternalInput")
    bg = nc.dram_tensor("bg", [128, 1], f32, kind="ExternalInput")
    bo = nc.dram_tensor("bo", [128, 2], f32, kind="ExternalInput")
    ident = nc.dram_tensor("ident", [128, 128], f32, kind="ExternalInput")
    y = nc.dram_tensor("y", [2, 128, 1024], f32, kind="ExternalOutput")

    with tile.TileContext(nc) as tc:
        with (
            tc.tile_pool(name="const", bufs=1) as cp,
            tc.tile_pool(name="big", bufs=1) as bp,
            tc.tile_pool(name="work", bufs=3) as wkp,
            tc.tile_pool(name="psum", bufs=1, space="PSUM") as pp,
        ):
            x0t = bp.tile([128, 4096], f16, tag="x0")
            x1t = bp.tile([128, 4096], f16, tag="x1")
            xs0t = bp.tile([128, 1024], f32, tag="xs0")
            xs1t = bp.tile([128, 1024], f32, tag="xs1")
            # critical startup loads fan out over four independent DMA
            # queues in 512-col chunks: each queue moves ~116 GB/s, so the
            # first convs (cols 0:512) unblock ~1.1us after issue and the
            # first attention pass (cols 0:1024) ~2.2us after
            nc.sync.dma_start(out=x0t[:, 0:512], in_=x0[:, 0:512])
            nc.gpsimd.dma_start(out=x1t[:, 0:512], in_=x1[:, 0:512])

            wct = cp.tile([128, 2, 256], f16, tag="wc")
            wgt = cp.tile([128, 2, 128], f16, tag="wg")
            wot = cp.tile([128, 2, 128], f16, tag="wo")
            btht = cp.tile([64, 128], f32, tag="bth")
            bpht = cp.tile([64, 128], f32, tag="bph")
            bgt = cp.tile([128, 1], f32, tag="bg")
            bot = cp.tile([128, 2], f32, tag="bo")
            idt = cp.tile([128, 128], f32, tag="ident")
            # only loads the prologue needs go on the queue up front; the
            # rest are emitted inside the main loop so the first h-pair's
            # DMA-semaphore wait clears as early as possible
            nc.scalar.dma_start(out=wct[:], in_=wc[:])
            nc.scalar.dma_start(out=btht[:], in_=bth[:])
            nc.scalar.dma_start(out=bpht[:], in_=bph[:])
            nc.scalar.dma_start(out=x0t[:, 512:1024], in_=x0[:, 512:1024])
            nc.gpsimd.dma_start(out=x1t[:, 512:1024], in_=x1[:, 512:1024])
            nc.gpsimd.dma_start(out=wgt[:], in_=wg[:])
            nc.sync.dma_start(out=bgt[:], in_=bg[:])
            for c0 in range(1024, 2048, 512):
                nc.sync.dma_start(out=x0t[:, c0:c0 + 512],
                                  in_=x0[:, c0:c0 + 512])
                nc.gpsimd.dma_start(out=x1t[:, c0:c0 + 512],
                                    in_=x1[:, c0:c0 + 512])

            def emit_late_dmas(step):
                if step == 0:
                    nc.sync.dma_start(out=x0t[:, 2048:4096],
                                      in_=x0[:, 2048:4096])
                    nc.sync.dma_start(out=x1t[:, 2048:4096],
                                      in_=x1[:, 2048:4096])
                elif step == 1:
                    nc.sync.dma_start(out=xs0t[:], in_=xs0[:])
                    nc.sync.dma_start(out=xs1t[:], in_=xs1[:])
                elif step == 2:
                    nc.sync.dma_start(out=wot[:], in_=wo[:])
                    nc.sync.dma_start(out=bot[:], in_=bo[:])
                    nc.sync.dma_start(out=idt[:], in_=ident[:])

            # preload the exp table set while DMAs/convs run, so the
            # first attention exp does not stall on ACT_TABLE_LOAD
            warm = wkp.tile([1, 1], f32, tag="warm")
            nc.scalar.activation(warm[:], btht[0:1, 0:1], AF.Exp)

            # [w; w] duplicated along partitions so the S matmul runs at
            # K=128: the HAM activity monitor never un-throttles the PE
            # clock (stays 1.2 GHz) for K=64 matmuls, measured directly.
            # The phi half of wc/bth is pre-scaled by 0.5 on the host so
            # the duplicated contraction sums to the original dot product.
            qt = bp.tile([128, LQ], f16, tag="qt")         # [Qt; Qt] (w2, q)
            pht = bp.tile([128, 8192], f16, tag="pht")     # [phT; phT]/2
            vaug = bp.tile([128, NKV * 65], bf16, tag="vaug")
            osb = bp.tile([128, 1024], f16, tag="osb")    # o (ch, (h, w))
            otsb = bp.tile([65, LQ], f32, tag="otsb")      # O'^T staged in SBUF
            ysb0 = bp.tile([128, 1024], f32, tag="ysb0")
            ysb1 = bp.tile([128, 1024], f32, tag="ysb1")
            xsb0 = bp.tile([128, 1024], f32, tag="xsb0")
            xsb1 = bp.tile([128, 1024], f32, tag="xsb1")
            vaug3 = vaug.rearrange("p (t j) -> p t j", j=65)
            nc.vector.memset(vaug3[:, :, 64:65], 1.0)

            # ---- producer emitters (interleaved into the attention loop so
            # the PE/DVE streams overlap attention instead of preceding it) --

            pht4 = pht.rearrange("p (g t c) -> p g t c", t=2, c=128)
            qt4 = qt.rearrange("p (g t c) -> p g t c", t=2, c=128)

            def emit_hp2(j):
                # Qt and phT columns for TWO h-pairs (hp = 2j, 2j+1) from x
                # via the [w_phi | w_theta] concatenated weights; batching two
                # h-pairs into one [128, 512] psum tile lets each evacuation
                # run as a single 3D-AP op over both (FD=256), halving the
                # DVE op count on the conv critical path
                hp0 = 2 * j
                n1 = 256 if hp0 < 8 else 128
                ps = pp.tile([128, 512], f32, tag="conv", bufs=2,
                             name=f"c{j}")
                for k in range(2):
                    hp = hp0 + k
                    nc.tensor.matmul(ps[:, k * 256:k * 256 + n1],
                                     lhsT=x0t[:, hp * 128:(hp + 1) * 128],
                                     rhs=wct[:, 0, 0:n1],
                                     start=True, stop=False)
                    nc.tensor.matmul(ps[:, k * 256:k * 256 + n1],
                                     lhsT=x1t[:, hp * 128:(hp + 1) * 128],
                                     rhs=wct[:, 1, 0:n1],
                                     start=False, stop=True)
                ps4 = ps.rearrange("p (k c) -> p k c", c=256)
                for hh in range(2):
                    # h = 2*hp + hh -> pht/qt col blocks at (4j+hh)*128,
                    # step 256 across the two hps of this batch
                    nc.vector.tensor_tensor(
                        out=pht4[0:64, 2 * j:2 * j + 2, hh, :],
                        in0=ps4[hh * 64:(hh + 1) * 64, :, 0:128],
                        in1=bpht[:].rearrange("p (o c) -> p o c", o=1)
                            .to_broadcast([64, 2, 128]),
                        op=ALU.add)
                    if hp0 < 8:
                        nc.vector.tensor_tensor(
                            out=qt4[0:64, 2 * j:2 * j + 2, hh, :],
                            in0=ps4[hh * 64:(hh + 1) * 64, :, 128:256],
                            in1=btht[:].rearrange("p (o c) -> p o c", o=1)
                                .to_broadcast([64, 2, 128]),
                            op=ALU.add)
                # duplicated w rows (partitions 64..127) via DMA -- no
                # compute-engine time; gpsimd's queue drains fast once the
                # x chunk stream finishes. Tiles 4j..4j+3 of pht.
                if j == 1:
                    nc.gpsimd.dma_start(out=qt[64:128, 0:1024],
                                        in_=qt[0:64, 0:1024])
                if j == 3:
                    nc.gpsimd.dma_start(out=qt[64:128, 1024:2048],
                                        in_=qt[0:64, 1024:2048])
                if j >= 1:
                    nc.gpsimd.dma_start(out=pht[64:128, j * 512:(j + 1) * 512],
                                        in_=pht[0:64, j * 512:(j + 1) * 512])

            def emit_g(n):
                # g conv chunk -> vaug (values only; ones column pre-memset)
                ps = pp.tile([128, 512], f32, tag="conv", bufs=2,
                             name=f"g{n}")
                nc.tensor.matmul(ps[:], lhsT=wgt[:, 0, :],
                                 rhs=x0t[:, n * 512:(n + 1) * 512],
                                 start=True, stop=False)
                nc.tensor.matmul(ps[:], lhsT=wgt[:, 1, :],
                                 rhs=x1t[:, n * 512:(n + 1) * 512],
                                 start=False, stop=True)
                nc.vector.tensor_scalar(
                    out=vaug3[:, n * 8:(n + 1) * 8, 0:64],
                    in0=ps.rearrange("p (t j) -> p t j", j=64)[:],
                    scalar1=bgt[:, 0:1], scalar2=None, op0=ALU.add)

            def emit_lh(lh, on_act):
                # transpose + normalize one 128-q block of O'^T
                trp = pp.tile([128, 65], f32, tag="conv", bufs=2,
                              name=f"tr{lh}")
                nc.tensor.transpose(trp[:],
                                    otsb[:, lh * 128:(lh + 1) * 128],
                                    idt[0:65, 0:65])
                rden = wkp.tile([128, 1], f32, tag="rden", name=f"rd{lh}")
                nc.vector.reciprocal(rden[:], trp[:, 64:65])
                if on_act:
                    nc.scalar.activation(osb[:, lh * 64:(lh + 1) * 64],
                                         trp[:, 0:64], AF.Copy,
                                         scale=rden[:])
                else:
                    nc.vector.tensor_scalar(
                        out=osb[:, lh * 64:(lh + 1) * 64],
                        in0=trp[:, 0:64],
                        scalar1=rden[:], scalar2=None, op0=ALU.mult)

            def emit_xsb(m):
                # precombine residual + out-conv bias while attention runs
                xsb = xsb0 if m == 0 else xsb1
                xres = xs0t if m == 0 else xs1t
                nc.vector.tensor_scalar(out=xsb[:], in0=xres[:],
                                        scalar1=bot[:, m:m + 1], scalar2=None,
                                        op0=ALU.add)

            def emit_y(m, c0, c1):
                # out conv for columns [c0:c1] + (bias+residual) + store
                ysb = ysb0 if m == 0 else ysb1
                xsb = xsb0 if m == 0 else xsb1
                # the conv psum pool is idle by the time any out-conv
                # runs; borrowing tag "s" here would steal an S double-buffer
                # slot mid-kernel and stall the attention pipeline
                yp = pp.tile([128, 512], f32, tag="conv", bufs=2,
                             name=f"yp{m}{c0}")
                nc.tensor.matmul(yp[:, 0:c1 - c0], lhsT=wot[:, m, :],
                                 rhs=osb[:, c0:c1], start=True, stop=True)
                nc.vector.tensor_tensor(
                    out=ysb[:, c0:c1], in0=yp[:, 0:c1 - c0],
                    in1=xsb[:, c0:c1], op=ALU.add)
                nc.sync.dma_start(out=y[m, :, c0:c1], in_=ysb[:, c0:c1])

            # ---- attention: software-pipelined over 2 q passes of 1024 ----
            NIT = 2 * NKV
            ets = {}
            otps = {}

            def emit_s(it):
                p, t = it // NKV, it % NKV
                # kv tiles 0-3 contract K=64 (no duplicated w rows -- those
                # rows aren't ready during startup); since phi is pre-halved
                # on the host for the K=128 duplicated contraction, the K=64
                # logits come out at 0.5x and the exp scale compensates.
                kk = 64 if t < 4 else 128
                sp = pp.tile([128, QP], f32, tag="s", bufs=2, name=f"sp{it}")
                for c in range(2):
                    nc.tensor.matmul(
                        sp[:, c * 512:(c + 1) * 512],
                        lhsT=pht[0:kk, t * 128:(t + 1) * 128],
                        rhs=qt[0:kk, p * QP + c * 512: p * QP + (c + 1) * 512],
                        start=True, stop=True)
                et = wkp.tile([128, QP], bf16, tag="e", bufs=4,
                              name=f"et{it}")
                esc = 2.0 if kk == 64 else 1.0
                if _dve_tile(it):
                    nc.vector.tensor_scalar(
                        out=et[:].bitcast(i16), in0=sp[:],
                        scalar1=SA * esc, scalar2=SB,
                        op0=ALU.mult, op1=ALU.add)
                else:
                    nc.scalar.activation(et[:], sp[:], AF.Exp, scale=esc)
                ets[it] = et

            def emit_pv(it):
                p, t = it // NKV, it % NKV
                if t == 0:
                    otps[p] = pp.tile([65, QP], f32, tag="ot", bufs=1,
                                      name=f"otp{p}")
                otp = otps[p]
                et = ets.pop(it)
                for c in range(2):
                    nc.tensor.matmul(
                        otp[:, c * 512:(c + 1) * 512],
                        lhsT=vaug3[:, t, :],
                        rhs=et[:, c * 512:(c + 1) * 512],
                        start=(t == 0), stop=(t == NKV - 1),
                        skip_group_check=True)
                if t == NKV - 1:
                    # evacuate O'^T in 256-col chunks on both free-ish
                    # engines so the downstream transpose/normalize chain
                    # starts as soon as its slice lands
                    for dc in range(4):
                        dst = otsb[:, p * QP + dc * 256:p * QP + (dc + 1) * 256]
                        src = otp[:, dc * 256:(dc + 1) * 256]
                        if dc % 2 == 0:
                            nc.vector.tensor_copy(dst, src)
                        else:
                            nc.scalar.activation(dst, src, AF.Copy)
                    if p == 0:
                        for lh in range(8):
                            todo.append(lambda lh=lh: emit_lh(lh, False))
                        todo.append(lambda: emit_xsb(0))
                        todo.append(lambda: emit_xsb(1))
                        todo.append(lambda: emit_y(0, 0, 512))
                        todo.append(lambda: emit_y(1, 0, 512))

            from collections import deque
            todo = deque()
            n_j = 2
            n_g = 1
            for j in range(n_j):
                emit_hp2(j)
            emit_s(0)
            emit_s(1)
            emit_g(0)
            for it in range(2, NIT):
                emit_s(it)
                emit_pv(it - 2)
                if it in (2, 4, 6):
                    emit_late_dmas(it // 2 - 1)
                if it % 3 == 0 and n_j < 16:
                    emit_hp2(n_j)
                    n_j += 1
                if it % 6 == 5 and n_g < 8:
                    emit_g(n_g)
                    n_g += 1
                if todo and it % 2 == 1:
                    todo.popleft()()
            emit_pv(NIT - 2)
            emit_pv(NIT - 1)
            while todo:
                todo.popleft()()
            for lh in range(8, 16):
                emit_lh(lh, True)
                if lh == 11:
                    emit_y(0, 512, 768)
                    emit_y(1, 512, 768)
            emit_y(0, 768, 1024)
            emit_y(1, 768, 1024)

    nc.compile()
    return nc


def _get_nc():
    if "nc" not in _cached:
        _cached["nc"] = _build_nc()
    return _cached["nc"]


LAST_EXEC_NS = None
LAST_TRACE_DIR = None


def kernel(inp, w_theta, b_theta, w_phi, b_phi, w_g, b_g, w_out, b_out):
    import os
    from concourse.bass_utils import run_bass_kernel_spmd

    nc = _get_nc()

    f = np.float32
    c = np.ascontiguousarray

    # [w_phi | w_theta] concatenated, as (c_lo, half, 256) fp16
    wcat = np.concatenate([w_phi.T * 0.5, w_theta.T], axis=1).astype(f)
    wc3 = c(wcat.reshape(2, 128, 256).transpose(1, 0, 2).astype(np.float16))
    wg3 = c(w_g.T.reshape(2, 128, CH).transpose(1, 0, 2).astype(np.float16))
    wo3 = c(w_out.reshape(2, 128, CH).transpose(2, 0, 1).astype(np.float16))  # [ch, m, co]
    bth1 = c(np.tile(b_theta.astype(f), (64, 1)))
    bph1 = c(np.tile(b_phi.astype(f) * 0.5, (64, 1)))
    bg1 = c(b_g.astype(f)[:, None])
    bo2 = c(b_out.reshape(2, 128).T.astype(f))
    ident = np.eye(128, dtype=f)

    in_maps = []
    for core in range(8):
        b, k = core // 4, core % 4
        x = inp[b].reshape(C, H, W).astype(f)
        # own h-block first, then the rest: kernel is h-order agnostic
        perm = list(range(HS * k, HS * (k + 1))) + \
            [h for h in range(H) if not (HS * k <= h < HS * (k + 1))]
        xp = x[:, perm, :].reshape(C, H * W)
        xp16 = xp.astype(np.float16)
        in_maps.append({
            "x0": c(xp16[:128]), "x1": c(xp16[128:]),
            "xs0": c(xp[:128, :1024]), "xs1": c(xp[128:, :1024]),
            "wc": wc3, "wg": wg3, "wo": wo3,
            "bth": bth1, "bph": bph1, "bg": bg1, "bo": bo2, "ident": ident,
        })

    trace = bool(os.environ.get("NLB_TRACE"))
    tmpdir = os.environ.get("NLB_TRACE_DIR") or None
    res = run_bass_kernel_spmd(nc, in_maps, list(range(8)), trace=trace,
                               tmpdir=tmpdir)
    global LAST_EXEC_NS, LAST_TRACE_DIR
    LAST_EXEC_NS = res.exec_time_ns
    LAST_TRACE_DIR = tmpdir

    out = np.empty((B, C, H, W), dtype=f)
    for core in range(8):
        b, k = core // 4, core % 4
        yc = res.results[core]["y"].reshape(C, HS, W)
        out[b, :, HS * k:HS * (k + 1), :] = yc
    return out

